# revision 57
# baseline (speedup 1.0000x reference)
"""Trainium2 Bass kernel for conditional-adjustment conv (CAConv), fp16.

Per sample b: h = relu(c[b] @ mlp_w1 + mlp_b1); adj = h @ mlp_w2 + mlp_b2;
w[b] = conv_w + adj.reshape(Co,Ci,3,3); out[b] = conv2d(x[b], w[b], pad=1) + conv_b.

Sharding: data-parallel over batch, 4 samples per core on 8 cores (SPMD).

All heavy matmuls in fp16 (full PE rate); psum accumulation stays fp32, so
rel err ~5e-4 << the 2e-2 budget. The host pre-casts padded x and the packed
w2 to fp16 and the kernel returns fp16 output (halves HBM traffic both
ways); the host casts back to fp32.

Per-core device kernel:
  Stage A (weight gen): four col-group fp32 matmuls (M=32 via
  zero-padded w1', tile_position=(0,32g)) compute the MLP hidden state
  for all 4 samples directly as a [128, 32] psum tile; one DVE fused
  add-bias+relu produces hT8 fp16 (col m = sample m%4, replicated at
  partition offsets 0/32/64/96 to match the packed w2). w2 is
  host-permuted to (ci, t, co) column order, packed in 4 k-groups
  [128, 9216] (group g = ci 16g..16g+16, rows 32g..32g+17), with
  mlp_b2 + conv_w folded into ones-row 16. For each 512-col chunk, 4
  matmuls with tile_position=(32g, 32g) write M=32 rows each so one
  psum tile [128, 512] is fully covered -> full-width DVE/ACT copies
  (fp32->fp16) into the partition-grouped adj4 [128, 9216] (row
  32g + 4r + b = sample b, ci group g, replica r).
  Weight placement is a two-hop scatter: (1) a 64-descriptor DMA per
  sample (partition-stride-32 source) into the compact staging tile
  wst[ci + 64*half, (t, co)]; (2) a same-partition strided DVE/ACT copy
  fans each half out onto the diagonal blocks of the per-pair
  block-diag tile wblk[ci + 64*half, t*128 + 64*half + co] (off-diag
  zeros from a DVE memset). This avoids the 576 tiny 128B descriptors
  per sample a direct scatter would need (measured ~16us of DMA-engine
  serialization gating conv start).
  Stage B (conv): host-padded fp16 x (130x130) for a sample pair lives
  as [ci(2 samples), h, w] across 128 partitions. Chunk-outer/
  tap-inner: each output chunk po[128, 512] (2 samples x 64 co
  partitions; 4 h-rows x 128 w free) accumulates its 9 shift-tap K=128
  fp16 matmuls back-to-back at the PE's 1 col/cycle peak (~218ns/MM),
  then its bias-copy (alternating DVE/ACT) fires immediately. Psum
  banks rotate through an explicit 8-tag round-robin ring (the pool's
  own slot picker reuses banks ~2 tiles apart, each reuse a ~1.3us PE
  stall on the copy drain). One output DMA per 16 h-rows; the final
  group's store is split so the tail is short. Plain junk matmuls pad
  the PE between stage A and conv so the HAM clock gate stays at
  full rate (col-tiled matmuls don't register as PE-busy to it).

  All DMAs ride the SP HWDGE queue (Q1) except output stores (ACT
  HWDGE, Q10): the arbiter gives Q1 strict priority, so bulk traffic
  parked on other queues starves. Ordering within Q1 is controlled
  with explicit sync deps: only the two xp0 chunks conv needs first
  precede the hop1 scatters; the remaining x backlog follows them.
  GPSIMD is avoided entirely (~10us ucode warm-up before its first op).
"""

import sys

if "/opt/trn_rl_repo" not in sys.path:
    sys.path.insert(0, "/opt/trn_rl_repo")

import numpy as np

B = 32
NCORES = 8
BPC = B // NCORES          # samples per core = 4
PAIRS = BPC // 2           # sample pairs per core = 2
CIN = COUT = 64
H = W = 128
HP = WP = 130              # padded dims
KH = KW = 3
NT = KH * KW               # taps = 9
CL = 8                     # c length
CL1 = CL + 1               # + ones row
MH = 16                    # mlp hidden
K2 = MH + 1                # mlp hidden + ones row
WTOT = NT * CIN * COUT     # 36864 weights per sample
GCOL = WTOT // 4           # 9216 cols per packed w2 group
XCH = 5                    # x chunks per pair
XCHE = (HP * WP) // XCH    # 3380 elems per chunk (26 padded rows)

_CACHE = {}


def _build():
    import concourse.bass as bass
    import concourse.mybir as mybir
    import concourse.tile as tile
    from concourse import bacc
    from concourse.tile_rust import add_dep_helper

    f32 = mybir.dt.float32
    f16 = mybir.dt.float16
    AF = mybir.ActivationFunctionType

    nc = bacc.Bacc("TRN2", target_bir_lowering=False, debug=False)

    xs_d = nc.dram_tensor("xsp", [BPC, CIN, HP * WP], f16, kind="ExternalInput")
    w2_d = nc.dram_tensor("w2p", [128, GCOL], f16, kind="ExternalInput")
    cst_d = nc.dram_tensor("cst", [128, 66], f32, kind="ExternalInput")
    out_d = nc.dram_tensor("out", [BPC, COUT, H, W], f16, kind="ExternalOutput")

    with tile.TileContext(nc) as tc:
        with (
            tc.tile_pool(name="consts", bufs=1) as consts,
            tc.tile_pool(name="adjpool", bufs=1) as adjpool,
            tc.tile_pool(name="xpool", bufs=2) as xpool,
            tc.tile_pool(name="opool", bufs=8) as opool,
            tc.tile_pool(name="pspool", bufs=1, space=bass.MemorySpace.PSUM) as ps,
        ):
            # ---- consts then packed w2 on the SP queue (gate stage A);
            # GPSIMD is avoided entirely: its DSPs take ~10us of ucode
            # load/drain before their first op ----
            # cst: cols 0-31 c'T tiled 8x (rows 0-8, ones row 8, col m =
            # sample m%4), 32-63 w1' zero-padded to M=32 (rows 0-8),
            # 64 b1 tiled at partition offsets 0/32/64/96, 65 conv_b x2
            cst = consts.tile([128, 66], f32)
            nc.sync.dma_start(out=cst[:], in_=cst_d.ap())
            # w2 128-row packed (rows 32g..32g+17 = group g): 2.36MB vs
            # 1.25MB dense, but a dense [17, *] load has only 17
            # descriptors per DMA and the engines assign per-descriptor
            # round-robin within a DMA — measured: the whole dense load
            # serialized onto ONE engine at ~55us. 128 descriptors/DMA
            # spread over all 16 engines beat the byte savings.
            w2s = consts.tile([128, GCOL], f16, name="w2s")
            # small leading chunk: stage A's first matmuls need only cols
            # 0-512, so don't make them wait for a whole 2304-col DMA
            w2cuts = [0, 512, GCOL // 4, GCOL // 2, 3 * GCOL // 4, GCOL]
            for c0, c1 in zip(w2cuts, w2cuts[1:]):
                nc.sync.dma_start(
                    out=w2s[:, c0:c1], in_=w2_d.ap()[:, c0:c1]
                )
            ct_sb = cst[0:CL1, 0:32]
            w1_sb = cst[0:CL1, 32:64]
            b1_sb = cst[:, 64:65]
            cb_sb = cst[:, 65:66]

            # psum bank ring: explicit round-robin via 8 single-buffer
            # tags — the pool's own slot picker reuses banks as little
            # as 1 tile apart, each reuse a ~1.3us PE stall on the copy
            # drain.
            psk = [0]

            def ptile(shape, name=None):
                t = ps.tile(shape, f32, tag=f"ps{psk[0] % 8}", bufs=1, name=name)
                psk[0] += 1
                return t

            # ---- stage A head: hT8 [128, 32] via one matmul quartet +
            # one DVE fused bias+relu (no ACT table, no replication
            # DMAs). Four col-group fp32 matmuls (M=32 via zero-padded
            # w1') fill all psum partitions; hT8 rows 32g+16 are the
            # ones row, rows 32g+17.. are relu(0)=0 and never read. ----
            ph32 = ptile([128, 32], name="ph32")
            for g in range(4):
                nc.tensor.matmul(
                    ph32[32 * g : 32 * g + 32, :], w1_sb, ct_sb,
                    start=True, stop=True, tile_position=(0, 32 * g),
                )
            ht8 = consts.tile([128, 32], f16, name="ht8")
            nc.vector.tensor_scalar(
                out=ht8[:], in0=ph32[:], scalar1=b1_sb, scalar2=0.0,
                op0=mybir.AluOpType.add, op1=mybir.AluOpType.max,
            )

            # ---- bulk x loads, all on Q1 (SP). The DMA arbiter gives
            # Q1 strict priority, so parking x on another queue starves
            # that queue instead of helping; the lever that works is
            # ORDER within the Q1 ring: only the two xp0 chunks conv
            # needs first go ahead of the hop1 scatters, the rest after.
            xps = []
            for p in range(PAIRS):
                xp = xpool.tile([128, HP * WP], f16, name=f"xp{p}", tag="xp")
                xps.append(xp)

            def load_x(p, ks, after=None):
                for k in ks:
                    inst = nc.sync.dma_start(
                        out=xps[p][:, k * XCHE : (k + 1) * XCHE],
                        in_=xs_d.ap()[2 * p : 2 * p + 2].rearrange(
                            "b c (k e) -> b c k e", e=XCHE
                        )[:, :, k, :],
                    )
                    if after is not None:
                        add_dep_helper(
                            inst.ins, after.ins, sync=True, reason="x after hop1"
                        )

            load_x(0, range(2))

            # per-pair block-diag weights; off-diag zeros via DVE memset
            # (DVE is idle early; GPSIMD wouldn't wake up for ~10us).
            # Emitted after the hT8 op so they don't head-block it in
            # the DVE FIFO.
            wblk = []
            for p in range(PAIRS):
                wb = consts.tile([128, NT * 128], f16, name=f"wblk{p}", tag=f"wblk{p}")
                nc.vector.memset(wb[:], 0.0)
                wblk.append(wb)

            # ---- stage A body: adj4[32g + 4r + b, c] = sample b's weight
            # for flat col 9216g + c ((ci,t,co) order), r = 0..7 replicas ----
            adj4 = adjpool.tile([128, GCOL], f16, name="adj4")
            for m in range(GCOL // 512):
                j = m * 512
                pa = ptile([128, 512])
                for g in range(4):
                    nc.tensor.matmul(
                        pa[32 * g : 32 * g + 32, :],
                        ht8[32 * g : 32 * g + K2, :],
                        w2s[32 * g : 32 * g + K2, j : j + 512],
                        start=True,
                        stop=True,
                        tile_position=(32 * g, 32 * g),
                    )
                if m == GCOL // 512 - 1:
                    # split the final chunk's copy across both engines:
                    # hop1 waits on all of adj4, so its last copy is on
                    # the critical path to conv start
                    nc.vector.tensor_copy(adj4[:, j : j + 256], pa[:, 0:256])
                    nc.scalar.copy(adj4[:, j + 256 : j + 512], pa[:, 256:512])
                elif m % 2 == 0:
                    nc.vector.tensor_copy(adj4[:, j : j + 512], pa[:])
                else:
                    nc.scalar.copy(adj4[:, j : j + 512], pa[:])

            # PE warm-up filler: the hop1->fanout handoff leaves the PE
            # idle long enough for the HAM clock gate to drop to half
            # rate and re-ramp over the first ~13 conv chunks. These
            # junk matmuls (results never read) keep it busy — plain
            # untiled ones: tile_position'd matmuls don't register as
            # PE-busy to the HAM (k=8/8 never engaged during stage A).
            for w in range(18):
                pw = ptile([128, 512], name=f"warm{w}")
                nc.tensor.matmul(
                    pw[0:32, :],
                    ht8[0:K2, :],
                    w2s[0:K2, 0:512],
                    start=True,
                    stop=True,
                )

            # two-hop weight placement (see module docstring)
            wst = []
            for p in range(PAIRS):
                wstp = consts.tile([128, NT * 64], f16, name=f"wst{p}", tag=f"wst{p}")
                wst.append(wstp)
            # hop1: pair 0 on Q1 (empty by now -> immediate); pair 1 on
            # Q10 ahead of xp1's descriptors (it only gates conv p1 at
            # ~85us). Fan-outs: pair-0 halves split DVE/ACT so wblk[0]
            # is ready ~2 copies after hop1 lands.
            def hop1(b, eng):
                p, half = divmod(b, 2)
                q = half * 64
                return eng.dma_start(
                    out=wst[p][q : q + 64, :], in_=adj4[b : 128 : 32, :]
                )

            def fanout(b, eng):
                p, half = divmod(b, 2)
                q = half * 64
                dst = wblk[p][q : q + 64, :].rearrange(
                    "p (t co) -> p t co", co=128
                )[:, :, q : q + 64]
                src = wst[p][q : q + 64, :].rearrange("p (t co) -> p t co", co=64)
                if eng is nc.vector:
                    eng.tensor_copy(dst, src)
                else:
                    eng.copy(dst, src)

            hop1_last = None
            for b in range(BPC):
                hop1_last = hop1(b, nc.sync)
            load_x(0, range(2, XCH), after=hop1_last)
            load_x(1, range(XCH), after=hop1_last)
            fanout(0, nc.vector)
            fanout(1, nc.scalar)
            fanout(2, nc.vector)
            fanout(3, nc.scalar)

            # ---- stage B: per-pair conv, chunk-outer / tap-inner ----
            for p in range(PAIRS):
                xp3 = xps[p].rearrange("p (h w) -> p h w", w=WP)
                for g in range(8):
                    os = opool.tile([128, 2048], f16, name=f"os{p}_{g}", tag="os")
                    for j in range(4):
                        h0 = (g * 4 + j) * 4
                        po = ptile([128, 512], name=f"po{p}_{g}_{j}")
                        for t in range(NT):
                            kh, kw = divmod(t, 3)
                            nc.tensor.matmul(
                                po[:],
                                wblk[p][:, t * 128 : (t + 1) * 128],
                                xp3[:, h0 + kh : h0 + kh + 4, kw : kw + W],
                                start=(t == 0),
                                stop=(t == NT - 1),
                            )
                        if j % 2 == 0:
                            nc.vector.tensor_scalar_add(
                                os[:, j * 512 : (j + 1) * 512], po[:], cb_sb
                            )
                        else:
                            nc.scalar.add(
                                os[:, j * 512 : (j + 1) * 512], po[:], cb_sb
                            )
                        if p == PAIRS - 1 and g == 7 and j == 1:
                            # split the final group's store so most of it
                            # streams while the last chunks compute
                            nc.scalar.dma_start(
                                out=out_d.ap()[
                                    2 * p : 2 * p + 2, :, 16 * g : 16 * g + 8, :
                                ],
                                in_=os[:, 0:1024],
                            )
                    if p == PAIRS - 1 and g == 7:
                        # final store from the idle SP queue so it isn't
                        # serialized behind the last ACT bias-copy
                        nc.sync.dma_start(
                            out=out_d.ap()[
                                2 * p : 2 * p + 2, :, 16 * g + 8 : 16 * g + 16, :
                            ],
                            in_=os[:, 1024:2048],
                        )
                    else:
                        nc.scalar.dma_start(
                            out=out_d.ap()[
                                2 * p : 2 * p + 2, :, 16 * g : 16 * g + 16, :
                            ],
                            in_=os[:],
                        )

    nc.compile()
    return nc


def _get_nc():
    if "nc" not in _CACHE:
        _CACHE["nc"] = _build()
    return _CACHE["nc"]


def _prep(x, c, conv_w, conv_b, mlp_w1, mlp_b1, mlp_w2, mlp_b2):
    x = np.asarray(x, dtype=np.float32)
    c = np.asarray(c, dtype=np.float32)
    conv_w = np.asarray(conv_w, dtype=np.float32)
    conv_b = np.asarray(conv_b, dtype=np.float32)
    mlp_w1 = np.asarray(mlp_w1, dtype=np.float32)
    mlp_b1 = np.asarray(mlp_b1, dtype=np.float32)
    mlp_w2 = np.asarray(mlp_w2, dtype=np.float32)
    mlp_b2 = np.asarray(mlp_b2, dtype=np.float32)

    # padded fp16 x, flattened spatial
    xsp = np.zeros((B, CIN, HP, WP), dtype=np.float16)
    xsp[:, :, 1 : HP - 1, 1 : WP - 1] = x.astype(np.float16)
    xsp = xsp.reshape(B, CIN, HP * WP)

    # w2p[k, (ci, t, co)] = mlp_w2[k, co*576 + ci*9 + t]
    # row 16 = (mlp_b2 + conv_w), same permutation -> adj == full weight
    w2p = mlp_w2.reshape(MH, COUT, CIN, NT).transpose(0, 2, 3, 1).reshape(MH, WTOT)
    b2p = mlp_b2.reshape(COUT, CIN, NT).transpose(1, 2, 0)
    cwp = conv_w.reshape(COUT, CIN, NT).transpose(1, 2, 0)  # [ci, t, co]
    row16 = (b2p + cwp).reshape(1, WTOT)
    w2p = np.concatenate([w2p, row16], axis=0)  # [17, 36864]
    # 4-group pack: rows 32g..32g+17 carry cols [9216g, 9216(g+1))
    w2pk = np.zeros((128, GCOL), dtype=np.float16)
    for g in range(4):
        w2pk[32 * g : 32 * g + K2] = w2p[:, GCOL * g : GCOL * (g + 1)].astype(
            np.float16
        )

    # packed consts [128, 66] f32 (core-invariant part): cols 0-31 c'T
    # tiled 8x, 32-63 w1' zero-padded to 32, 64 b1 tiled per k-group,
    # 65 conv_b x2
    cstb = np.zeros((128, 66), dtype=np.float32)
    cstb[:CL, 32 : 32 + MH] = mlp_w1
    cstb[CL, 32 + MH] = 1.0
    for g in range(4):
        cstb[32 * g : 32 * g + MH, 64] = mlp_b1
    cstb[:, 65] = np.tile(conv_b, 2)

    in_maps = []
    for i in range(NCORES):
        sl = slice(i * BPC, (i + 1) * BPC)
        cst = cstb.copy()
        cst[:CL, 0:32] = np.tile(c[sl].T, (1, 8))
        cst[CL, 0:32] = 1.0
        in_maps.append(
            {
                "xsp": np.ascontiguousarray(xsp[sl]),
                "w2p": w2pk,
                "cst": cst,
            }
        )
    return in_maps


def _run(inputs, trace=False):
    from concourse.bass_utils import run_bass_kernel_spmd

    nc = _get_nc()
    in_maps = _prep(**inputs)
    res = run_bass_kernel_spmd(
        nc, in_maps, core_ids=list(range(NCORES)), trace=trace
    )
    out = np.concatenate(
        [res.results[i]["out"].astype(np.float32) for i in range(NCORES)], axis=0
    )
    return out, res


def kernel(**inputs):
    out, _ = _run(inputs, trace=False)
    return out


# revision 58
# speedup vs baseline: 1.0033x; 1.0033x over previous
"""Trainium2 Bass kernel for conditional-adjustment conv (CAConv), fp16.

Per sample b: h = relu(c[b] @ mlp_w1 + mlp_b1); adj = h @ mlp_w2 + mlp_b2;
w[b] = conv_w + adj.reshape(Co,Ci,3,3); out[b] = conv2d(x[b], w[b], pad=1) + conv_b.

Sharding: data-parallel over batch, 4 samples per core on 8 cores (SPMD).

All heavy matmuls in fp16 (full PE rate); psum accumulation stays fp32, so
rel err ~5e-4 << the 2e-2 budget. The host pre-casts padded x and the packed
w2 to fp16 and the kernel returns fp16 output (halves HBM traffic both
ways); the host casts back to fp32.

Per-core device kernel:
  Stage A (weight gen): four col-group fp32 matmuls (M=32 via
  zero-padded w1', tile_position=(0,32g)) compute the MLP hidden state
  for all 4 samples directly as a [128, 32] psum tile; one DVE fused
  add-bias+relu produces hT8 fp16 (col m = sample m%4, replicated at
  partition offsets 0/32/64/96 to match the packed w2). w2 is
  host-permuted to (ci, t, co) column order, packed in 4 k-groups
  [128, 9216] (group g = ci 16g..16g+16, rows 32g..32g+17), with
  mlp_b2 + conv_w folded into ones-row 16. For each 512-col chunk, 4
  matmuls with tile_position=(32g, 32g) write M=32 rows each so one
  psum tile [128, 512] is fully covered -> full-width DVE/ACT copies
  (fp32->fp16) into the partition-grouped adj4 [128, 9216] (row
  32g + 4r + b = sample b, ci group g, replica r).
  Weight placement is a two-hop scatter: (1) a 64-descriptor DMA per
  sample (partition-stride-32 source) into the compact staging tile
  wst[ci + 64*half, (t, co)]; (2) a same-partition strided DVE/ACT copy
  fans each half out onto the diagonal blocks of the per-pair
  block-diag tile wblk[ci + 64*half, t*128 + 64*half + co] (off-diag
  zeros from a DVE memset). This avoids the 576 tiny 128B descriptors
  per sample a direct scatter would need (measured ~16us of DMA-engine
  serialization gating conv start).
  Stage B (conv): host-padded fp16 x (130x130) for a sample pair lives
  as [ci(2 samples), h, w] across 128 partitions. Chunk-outer/
  tap-inner: each output chunk po[128, 512] (2 samples x 64 co
  partitions; 4 h-rows x 128 w free) accumulates its 9 shift-tap K=128
  fp16 matmuls back-to-back at the PE's 1 col/cycle peak (~218ns/MM),
  then its bias-copy (alternating DVE/ACT) fires immediately. Psum
  banks rotate through an explicit 8-tag round-robin ring (the pool's
  own slot picker reuses banks ~2 tiles apart, each reuse a ~1.3us PE
  stall on the copy drain). One output DMA per 16 h-rows; the final
  group's store is split so the tail is short. Plain junk matmuls pad
  the PE between stage A and conv so the HAM clock gate stays at
  full rate (col-tiled matmuls don't register as PE-busy to it).

  All DMAs ride the SP HWDGE queue (Q1) except output stores (ACT
  HWDGE, Q10): the arbiter gives Q1 strict priority, so bulk traffic
  parked on other queues starves. Ordering within Q1 is controlled
  with explicit sync deps: only the two xp0 chunks conv needs first
  precede the hop1 scatters; the remaining x backlog follows them.
  GPSIMD is avoided entirely (~10us ucode warm-up before its first op).
"""

import sys

if "/opt/trn_rl_repo" not in sys.path:
    sys.path.insert(0, "/opt/trn_rl_repo")

import numpy as np

B = 32
NCORES = 8
BPC = B // NCORES          # samples per core = 4
PAIRS = BPC // 2           # sample pairs per core = 2
CIN = COUT = 64
H = W = 128
HP = WP = 130              # padded dims
KH = KW = 3
NT = KH * KW               # taps = 9
CL = 8                     # c length
CL1 = CL + 1               # + ones row
MH = 16                    # mlp hidden
K2 = MH + 1                # mlp hidden + ones row
WTOT = NT * CIN * COUT     # 36864 weights per sample
GCOL = WTOT // 4           # 9216 cols per packed w2 group
XCH = 5                    # x chunks per pair
XCHE = (HP * WP) // XCH    # 3380 elems per chunk (26 padded rows)

_CACHE = {}


def _build():
    import concourse.bass as bass
    import concourse.mybir as mybir
    import concourse.tile as tile
    from concourse import bacc
    from concourse.tile_rust import add_dep_helper

    f32 = mybir.dt.float32
    f16 = mybir.dt.float16
    AF = mybir.ActivationFunctionType

    nc = bacc.Bacc("TRN2", target_bir_lowering=False, debug=False)

    xs_d = nc.dram_tensor("xsp", [BPC, CIN, HP * WP], f16, kind="ExternalInput")
    w2_d = nc.dram_tensor("w2p", [128, GCOL], f16, kind="ExternalInput")
    cst_d = nc.dram_tensor("cst", [128, 66], f32, kind="ExternalInput")
    out_d = nc.dram_tensor("out", [BPC, COUT, H, W], f16, kind="ExternalOutput")

    with tile.TileContext(nc) as tc:
        with (
            tc.tile_pool(name="consts", bufs=1) as consts,
            tc.tile_pool(name="adjpool", bufs=1) as adjpool,
            tc.tile_pool(name="xpool", bufs=2) as xpool,
            tc.tile_pool(name="opool", bufs=8) as opool,
            tc.tile_pool(name="pspool", bufs=1, space=bass.MemorySpace.PSUM) as ps,
        ):
            # ---- consts then packed w2 on the SP queue (gate stage A);
            # GPSIMD is avoided entirely: its DSPs take ~10us of ucode
            # load/drain before their first op ----
            # cst: cols 0-31 c'T tiled 8x (rows 0-8, ones row 8, col m =
            # sample m%4), 32-63 w1' zero-padded to M=32 (rows 0-8),
            # 64 b1 tiled at partition offsets 0/32/64/96, 65 conv_b x2
            cst = consts.tile([128, 66], f32)
            nc.sync.dma_start(out=cst[:], in_=cst_d.ap())
            # w2 128-row packed (rows 32g..32g+17 = group g): 2.36MB vs
            # 1.25MB dense, but a dense [17, *] load has only 17
            # descriptors per DMA and the engines assign per-descriptor
            # round-robin within a DMA — measured: the whole dense load
            # serialized onto ONE engine at ~55us. 128 descriptors/DMA
            # spread over all 16 engines beat the byte savings.
            w2s = consts.tile([128, GCOL], f16, name="w2s")
            # small leading chunk: stage A's first matmuls need only cols
            # 0-512, so don't make them wait for a whole 2304-col DMA
            w2cuts = [0, 512, GCOL // 4, GCOL // 2, 3 * GCOL // 4, GCOL]
            for c0, c1 in zip(w2cuts, w2cuts[1:]):
                nc.sync.dma_start(
                    out=w2s[:, c0:c1], in_=w2_d.ap()[:, c0:c1]
                )
            ct_sb = cst[0:CL1, 0:32]
            w1_sb = cst[0:CL1, 32:64]
            b1_sb = cst[:, 64:65]
            cb_sb = cst[:, 65:66]

            # psum bank ring: explicit round-robin via 8 single-buffer
            # tags — the pool's own slot picker reuses banks as little
            # as 1 tile apart, each reuse a ~1.3us PE stall on the copy
            # drain.
            psk = [0]

            def ptile(shape, name=None):
                t = ps.tile(shape, f32, tag=f"ps{psk[0] % 8}", bufs=1, name=name)
                psk[0] += 1
                return t

            # ---- stage A head: hT8 [128, 32] via one matmul quartet +
            # one DVE fused bias+relu (no ACT table, no replication
            # DMAs). Four col-group fp32 matmuls (M=32 via zero-padded
            # w1') fill all psum partitions; hT8 rows 32g+16 are the
            # ones row, rows 32g+17.. are relu(0)=0 and never read. ----
            ph32 = ptile([128, 32], name="ph32")
            for g in range(4):
                nc.tensor.matmul(
                    ph32[32 * g : 32 * g + 32, :], w1_sb, ct_sb,
                    start=True, stop=True, tile_position=(0, 32 * g),
                )
            ht8 = consts.tile([128, 32], f16, name="ht8")
            nc.vector.tensor_scalar(
                out=ht8[:], in0=ph32[:], scalar1=b1_sb, scalar2=0.0,
                op0=mybir.AluOpType.add, op1=mybir.AluOpType.max,
            )

            # ---- bulk x loads, all on Q1 (SP). The DMA arbiter gives
            # Q1 strict priority, so parking x on another queue starves
            # that queue instead of helping; the lever that works is
            # ORDER within the Q1 ring: only the two xp0 chunks conv
            # needs first go ahead of the hop1 scatters, the rest after.
            xps = []
            for p in range(PAIRS):
                xp = xpool.tile([128, HP * WP], f16, name=f"xp{p}", tag="xp")
                xps.append(xp)

            def load_x(p, ks, after=None):
                for k in ks:
                    inst = nc.sync.dma_start(
                        out=xps[p][:, k * XCHE : (k + 1) * XCHE],
                        in_=xs_d.ap()[2 * p : 2 * p + 2].rearrange(
                            "b c (k e) -> b c k e", e=XCHE
                        )[:, :, k, :],
                    )
                    if after is not None:
                        add_dep_helper(
                            inst.ins, after.ins, sync=True, reason="x after hop1"
                        )

            load_x(0, range(2))

            # per-pair block-diag weights; off-diag zeros via DVE memset
            # (DVE is idle early; GPSIMD wouldn't wake up for ~10us).
            # Emitted after the hT8 op so they don't head-block it in
            # the DVE FIFO.
            wblk = []
            for p in range(PAIRS):
                wb = consts.tile([128, NT * 128], f16, name=f"wblk{p}", tag=f"wblk{p}")
                nc.vector.memset(wb[:], 0.0)
                wblk.append(wb)

            # ---- stage A body: adj4[32g + 4r + b, c] = sample b's weight
            # for flat col 9216g + c ((ci,t,co) order), r = 0..7 replicas ----
            adj4 = adjpool.tile([128, GCOL], f16, name="adj4")
            for m in range(GCOL // 512):
                j = m * 512
                pa = ptile([128, 512])
                for g in range(4):
                    nc.tensor.matmul(
                        pa[32 * g : 32 * g + 32, :],
                        ht8[32 * g : 32 * g + K2, :],
                        w2s[32 * g : 32 * g + K2, j : j + 512],
                        start=True,
                        stop=True,
                        tile_position=(32 * g, 32 * g),
                    )
                if m == GCOL // 512 - 1:
                    # split the final chunk's copy across both engines:
                    # hop1 waits on all of adj4, so its last copy is on
                    # the critical path to conv start
                    nc.vector.tensor_copy(adj4[:, j : j + 256], pa[:, 0:256])
                    nc.scalar.copy(adj4[:, j + 256 : j + 512], pa[:, 256:512])
                elif m % 2 == 0:
                    nc.vector.tensor_copy(adj4[:, j : j + 512], pa[:])
                else:
                    nc.scalar.copy(adj4[:, j : j + 512], pa[:])

            # PE warm-up filler: the hop1->fanout handoff leaves the PE
            # idle long enough for the HAM clock gate to drop to half
            # rate and re-ramp over the first ~13 conv chunks. These
            # junk matmuls (results never read) keep it busy — plain
            # untiled ones: tile_position'd matmuls don't register as
            # PE-busy to the HAM (k=8/8 never engaged during stage A).
            for w in range(10):
                pw = ptile([128, 512], name=f"warm{w}")
                nc.tensor.matmul(
                    pw[0:32, :],
                    ht8[0:K2, :],
                    w2s[0:K2, 0:512],
                    start=True,
                    stop=True,
                )

            # two-hop weight placement (see module docstring)
            wst = []
            for p in range(PAIRS):
                wstp = consts.tile([128, NT * 64], f16, name=f"wst{p}", tag=f"wst{p}")
                wst.append(wstp)
            # hop1: pair 0 on Q1 (empty by now -> immediate); pair 1 on
            # Q10 ahead of xp1's descriptors (it only gates conv p1 at
            # ~85us). Fan-outs: pair-0 halves split DVE/ACT so wblk[0]
            # is ready ~2 copies after hop1 lands.
            def hop1(b, eng):
                p, half = divmod(b, 2)
                q = half * 64
                return eng.dma_start(
                    out=wst[p][q : q + 64, :], in_=adj4[b : 128 : 32, :]
                )

            def fanout(b, eng):
                p, half = divmod(b, 2)
                q = half * 64
                dst = wblk[p][q : q + 64, :].rearrange(
                    "p (t co) -> p t co", co=128
                )[:, :, q : q + 64]
                src = wst[p][q : q + 64, :].rearrange("p (t co) -> p t co", co=64)
                if eng is nc.vector:
                    eng.tensor_copy(dst, src)
                else:
                    eng.copy(dst, src)

            hop1_last = None
            for b in range(BPC):
                hop1_last = hop1(b, nc.sync)
            load_x(0, range(2, XCH), after=hop1_last)
            load_x(1, range(XCH), after=hop1_last)
            fanout(0, nc.vector)
            fanout(1, nc.scalar)
            fanout(2, nc.vector)
            fanout(3, nc.scalar)

            # ---- stage B: per-pair conv, chunk-outer / tap-inner ----
            for p in range(PAIRS):
                xp3 = xps[p].rearrange("p (h w) -> p h w", w=WP)
                for g in range(8):
                    os = opool.tile([128, 2048], f16, name=f"os{p}_{g}", tag="os")
                    for j in range(4):
                        h0 = (g * 4 + j) * 4
                        po = ptile([128, 512], name=f"po{p}_{g}_{j}")
                        for t in range(NT):
                            kh, kw = divmod(t, 3)
                            nc.tensor.matmul(
                                po[:],
                                wblk[p][:, t * 128 : (t + 1) * 128],
                                xp3[:, h0 + kh : h0 + kh + 4, kw : kw + W],
                                start=(t == 0),
                                stop=(t == NT - 1),
                            )
                        if j % 2 == 0:
                            nc.vector.tensor_scalar_add(
                                os[:, j * 512 : (j + 1) * 512], po[:], cb_sb
                            )
                        else:
                            nc.scalar.add(
                                os[:, j * 512 : (j + 1) * 512], po[:], cb_sb
                            )
                        if p == PAIRS - 1 and g == 7 and j == 1:
                            # split the final group's store so most of it
                            # streams while the last chunks compute
                            nc.scalar.dma_start(
                                out=out_d.ap()[
                                    2 * p : 2 * p + 2, :, 16 * g : 16 * g + 8, :
                                ],
                                in_=os[:, 0:1024],
                            )
                    if p == PAIRS - 1 and g == 7:
                        # final store from the idle SP queue so it isn't
                        # serialized behind the last ACT bias-copy
                        nc.sync.dma_start(
                            out=out_d.ap()[
                                2 * p : 2 * p + 2, :, 16 * g + 8 : 16 * g + 16, :
                            ],
                            in_=os[:, 1024:2048],
                        )
                    else:
                        nc.scalar.dma_start(
                            out=out_d.ap()[
                                2 * p : 2 * p + 2, :, 16 * g : 16 * g + 16, :
                            ],
                            in_=os[:],
                        )

    nc.compile()
    return nc


def _get_nc():
    if "nc" not in _CACHE:
        _CACHE["nc"] = _build()
    return _CACHE["nc"]


def _prep(x, c, conv_w, conv_b, mlp_w1, mlp_b1, mlp_w2, mlp_b2):
    x = np.asarray(x, dtype=np.float32)
    c = np.asarray(c, dtype=np.float32)
    conv_w = np.asarray(conv_w, dtype=np.float32)
    conv_b = np.asarray(conv_b, dtype=np.float32)
    mlp_w1 = np.asarray(mlp_w1, dtype=np.float32)
    mlp_b1 = np.asarray(mlp_b1, dtype=np.float32)
    mlp_w2 = np.asarray(mlp_w2, dtype=np.float32)
    mlp_b2 = np.asarray(mlp_b2, dtype=np.float32)

    # padded fp16 x, flattened spatial
    xsp = np.zeros((B, CIN, HP, WP), dtype=np.float16)
    xsp[:, :, 1 : HP - 1, 1 : WP - 1] = x.astype(np.float16)
    xsp = xsp.reshape(B, CIN, HP * WP)

    # w2p[k, (ci, t, co)] = mlp_w2[k, co*576 + ci*9 + t]
    # row 16 = (mlp_b2 + conv_w), same permutation -> adj == full weight
    w2p = mlp_w2.reshape(MH, COUT, CIN, NT).transpose(0, 2, 3, 1).reshape(MH, WTOT)
    b2p = mlp_b2.reshape(COUT, CIN, NT).transpose(1, 2, 0)
    cwp = conv_w.reshape(COUT, CIN, NT).transpose(1, 2, 0)  # [ci, t, co]
    row16 = (b2p + cwp).reshape(1, WTOT)
    w2p = np.concatenate([w2p, row16], axis=0)  # [17, 36864]
    # 4-group pack: rows 32g..32g+17 carry cols [9216g, 9216(g+1))
    w2pk = np.zeros((128, GCOL), dtype=np.float16)
    for g in range(4):
        w2pk[32 * g : 32 * g + K2] = w2p[:, GCOL * g : GCOL * (g + 1)].astype(
            np.float16
        )

    # packed consts [128, 66] f32 (core-invariant part): cols 0-31 c'T
    # tiled 8x, 32-63 w1' zero-padded to 32, 64 b1 tiled per k-group,
    # 65 conv_b x2
    cstb = np.zeros((128, 66), dtype=np.float32)
    cstb[:CL, 32 : 32 + MH] = mlp_w1
    cstb[CL, 32 + MH] = 1.0
    for g in range(4):
        cstb[32 * g : 32 * g + MH, 64] = mlp_b1
    cstb[:, 65] = np.tile(conv_b, 2)

    in_maps = []
    for i in range(NCORES):
        sl = slice(i * BPC, (i + 1) * BPC)
        cst = cstb.copy()
        cst[:CL, 0:32] = np.tile(c[sl].T, (1, 8))
        cst[CL, 0:32] = 1.0
        in_maps.append(
            {
                "xsp": np.ascontiguousarray(xsp[sl]),
                "w2p": w2pk,
                "cst": cst,
            }
        )
    return in_maps


def _run(inputs, trace=False):
    from concourse.bass_utils import run_bass_kernel_spmd

    nc = _get_nc()
    in_maps = _prep(**inputs)
    res = run_bass_kernel_spmd(
        nc, in_maps, core_ids=list(range(NCORES)), trace=trace
    )
    out = np.concatenate(
        [res.results[i]["out"].astype(np.float32) for i in range(NCORES)], axis=0
    )
    return out, res


def kernel(**inputs):
    out, _ = _run(inputs, trace=False)
    return out


# revision 59
# speedup vs baseline: 1.0042x; 1.0009x over previous
"""Trainium2 Bass kernel for conditional-adjustment conv (CAConv), fp16.

Per sample b: h = relu(c[b] @ mlp_w1 + mlp_b1); adj = h @ mlp_w2 + mlp_b2;
w[b] = conv_w + adj.reshape(Co,Ci,3,3); out[b] = conv2d(x[b], w[b], pad=1) + conv_b.

Sharding: data-parallel over batch, 4 samples per core on 8 cores (SPMD).

All heavy matmuls in fp16 (full PE rate); psum accumulation stays fp32, so
rel err ~5e-4 << the 2e-2 budget. The host pre-casts padded x and the packed
w2 to fp16 and the kernel returns fp16 output (halves HBM traffic both
ways); the host casts back to fp32.

Per-core device kernel:
  Stage A (weight gen): four col-group fp32 matmuls (M=32 via
  zero-padded w1', tile_position=(0,32g)) compute the MLP hidden state
  for all 4 samples directly as a [128, 32] psum tile; one DVE fused
  add-bias+relu produces hT8 fp16 (col m = sample m%4, replicated at
  partition offsets 0/32/64/96 to match the packed w2). w2 is
  host-permuted to (ci, t, co) column order, packed in 4 k-groups
  [128, 9216] (group g = ci 16g..16g+16, rows 32g..32g+17), with
  mlp_b2 + conv_w folded into ones-row 16. For each 512-col chunk, 4
  matmuls with tile_position=(32g, 32g) write M=32 rows each so one
  psum tile [128, 512] is fully covered -> full-width DVE/ACT copies
  (fp32->fp16) into the partition-grouped adj4 [128, 9216] (row
  32g + 4r + b = sample b, ci group g, replica r).
  Weight placement is a two-hop scatter: (1) a 64-descriptor DMA per
  sample (partition-stride-32 source) into the compact staging tile
  wst[ci + 64*half, (t, co)]; (2) a same-partition strided DVE/ACT copy
  fans each half out onto the diagonal blocks of the per-pair
  block-diag tile wblk[ci + 64*half, t*128 + 64*half + co] (off-diag
  zeros from a DVE memset). This avoids the 576 tiny 128B descriptors
  per sample a direct scatter would need (measured ~16us of DMA-engine
  serialization gating conv start).
  Stage B (conv): host-padded fp16 x (130x130) for a sample pair lives
  as [ci(2 samples), h, w] across 128 partitions. Chunk-outer/
  tap-inner: each output chunk po[128, 512] (2 samples x 64 co
  partitions; 4 h-rows x 128 w free) accumulates its 9 shift-tap K=128
  fp16 matmuls back-to-back at the PE's 1 col/cycle peak (~218ns/MM),
  then its bias-copy (alternating DVE/ACT) fires immediately. Psum
  banks rotate through an explicit 8-tag round-robin ring (the pool's
  own slot picker reuses banks ~2 tiles apart, each reuse a ~1.3us PE
  stall on the copy drain). One output DMA per 16 h-rows; the final
  group's store is split so the tail is short. Plain junk matmuls pad
  the PE between stage A and conv so the HAM clock gate stays at
  full rate (col-tiled matmuls don't register as PE-busy to it).

  All DMAs ride the SP HWDGE queue (Q1) except output stores (ACT
  HWDGE, Q10): the arbiter gives Q1 strict priority, so bulk traffic
  parked on other queues starves. Ordering within Q1 is controlled
  with explicit sync deps: only the two xp0 chunks conv needs first
  precede the hop1 scatters; the remaining x backlog follows them.
  GPSIMD is avoided entirely (~10us ucode warm-up before its first op).
"""

import sys

if "/opt/trn_rl_repo" not in sys.path:
    sys.path.insert(0, "/opt/trn_rl_repo")

import numpy as np

B = 32
NCORES = 8
BPC = B // NCORES          # samples per core = 4
PAIRS = BPC // 2           # sample pairs per core = 2
CIN = COUT = 64
H = W = 128
HP = WP = 130              # padded dims
KH = KW = 3
NT = KH * KW               # taps = 9
CL = 8                     # c length
CL1 = CL + 1               # + ones row
MH = 16                    # mlp hidden
K2 = MH + 1                # mlp hidden + ones row
WTOT = NT * CIN * COUT     # 36864 weights per sample
GCOL = WTOT // 4           # 9216 cols per packed w2 group
XCH = 5                    # x chunks per pair
XCHE = (HP * WP) // XCH    # 3380 elems per chunk (26 padded rows)

_CACHE = {}


def _build():
    import concourse.bass as bass
    import concourse.mybir as mybir
    import concourse.tile as tile
    from concourse import bacc
    from concourse.tile_rust import add_dep_helper

    f32 = mybir.dt.float32
    f16 = mybir.dt.float16
    AF = mybir.ActivationFunctionType

    nc = bacc.Bacc("TRN2", target_bir_lowering=False, debug=False)

    xs_d = nc.dram_tensor("xsp", [BPC, CIN, HP * WP], f16, kind="ExternalInput")
    w2_d = nc.dram_tensor("w2p", [128, GCOL], f16, kind="ExternalInput")
    cst_d = nc.dram_tensor("cst", [128, 66], f32, kind="ExternalInput")
    out_d = nc.dram_tensor("out", [BPC, COUT, H, W], f16, kind="ExternalOutput")

    with tile.TileContext(nc) as tc:
        with (
            tc.tile_pool(name="consts", bufs=1) as consts,
            tc.tile_pool(name="adjpool", bufs=1) as adjpool,
            tc.tile_pool(name="xpool", bufs=2) as xpool,
            tc.tile_pool(name="opool", bufs=8) as opool,
            tc.tile_pool(name="pspool", bufs=1, space=bass.MemorySpace.PSUM) as ps,
        ):
            # ---- consts then packed w2 on the SP queue (gate stage A);
            # GPSIMD is avoided entirely: its DSPs take ~10us of ucode
            # load/drain before their first op ----
            # cst: cols 0-31 c'T tiled 8x (rows 0-8, ones row 8, col m =
            # sample m%4), 32-63 w1' zero-padded to M=32 (rows 0-8),
            # 64 b1 tiled at partition offsets 0/32/64/96, 65 conv_b x2
            cst = consts.tile([128, 66], f32)
            nc.sync.dma_start(out=cst[:], in_=cst_d.ap())
            # w2 128-row packed (rows 32g..32g+17 = group g): 2.36MB vs
            # 1.25MB dense, but a dense [17, *] load has only 17
            # descriptors per DMA and the engines assign per-descriptor
            # round-robin within a DMA — measured: the whole dense load
            # serialized onto ONE engine at ~55us. 128 descriptors/DMA
            # spread over all 16 engines beat the byte savings.
            w2s = consts.tile([128, GCOL], f16, name="w2s")
            # small leading chunk: stage A's first matmuls need only cols
            # 0-512, so don't make them wait for a whole 2304-col DMA
            w2cuts = [0, 512, GCOL // 4, GCOL // 2, 3 * GCOL // 4, GCOL]
            for c0, c1 in zip(w2cuts, w2cuts[1:]):
                nc.sync.dma_start(
                    out=w2s[:, c0:c1], in_=w2_d.ap()[:, c0:c1]
                )
            ct_sb = cst[0:CL1, 0:32]
            w1_sb = cst[0:CL1, 32:64]
            b1_sb = cst[:, 64:65]
            cb_sb = cst[:, 65:66]

            # psum bank ring: explicit round-robin via 8 single-buffer
            # tags — the pool's own slot picker reuses banks as little
            # as 1 tile apart, each reuse a ~1.3us PE stall on the copy
            # drain.
            psk = [0]

            def ptile(shape, name=None):
                t = ps.tile(shape, f32, tag=f"ps{psk[0] % 8}", bufs=1, name=name)
                psk[0] += 1
                return t

            # ---- stage A head: hT8 [128, 32] via one matmul quartet +
            # one DVE fused bias+relu (no ACT table, no replication
            # DMAs). Four col-group fp32 matmuls (M=32 via zero-padded
            # w1') fill all psum partitions; hT8 rows 32g+16 are the
            # ones row, rows 32g+17.. are relu(0)=0 and never read. ----
            ph32 = ptile([128, 32], name="ph32")
            for g in range(4):
                nc.tensor.matmul(
                    ph32[32 * g : 32 * g + 32, :], w1_sb, ct_sb,
                    start=True, stop=True, tile_position=(0, 32 * g),
                )
            ht8 = consts.tile([128, 32], f16, name="ht8")
            nc.vector.tensor_scalar(
                out=ht8[:], in0=ph32[:], scalar1=b1_sb, scalar2=0.0,
                op0=mybir.AluOpType.add, op1=mybir.AluOpType.max,
            )

            # ---- bulk x loads, all on Q1 (SP). The DMA arbiter gives
            # Q1 strict priority, so parking x on another queue starves
            # that queue instead of helping; the lever that works is
            # ORDER within the Q1 ring: only the two xp0 chunks conv
            # needs first go ahead of the hop1 scatters, the rest after.
            xps = []
            for p in range(PAIRS):
                xp = xpool.tile([128, HP * WP], f16, name=f"xp{p}", tag="xp")
                xps.append(xp)

            def load_x(p, ks, after=None):
                for k in ks:
                    inst = nc.sync.dma_start(
                        out=xps[p][:, k * XCHE : (k + 1) * XCHE],
                        in_=xs_d.ap()[2 * p : 2 * p + 2].rearrange(
                            "b c (k e) -> b c k e", e=XCHE
                        )[:, :, k, :],
                    )
                    if after is not None:
                        add_dep_helper(
                            inst.ins, after.ins, sync=True, reason="x after hop1"
                        )

            load_x(0, range(2))

            # per-pair block-diag weights; off-diag zeros via DVE memset
            # (DVE is idle early; GPSIMD wouldn't wake up for ~10us).
            # Emitted after the hT8 op so they don't head-block it in
            # the DVE FIFO.
            wblk = []
            for p in range(PAIRS):
                wb = consts.tile([128, NT * 128], f16, name=f"wblk{p}", tag=f"wblk{p}")
                nc.vector.memset(wb[:], 0.0)
                wblk.append(wb)

            # ---- stage A body: adj4[32g + 4r + b, c] = sample b's weight
            # for flat col 9216g + c ((ci,t,co) order), r = 0..7 replicas ----
            adj4 = adjpool.tile([128, GCOL], f16, name="adj4")
            for m in range(GCOL // 512):
                j = m * 512
                pa = ptile([128, 512])
                for g in range(4):
                    nc.tensor.matmul(
                        pa[32 * g : 32 * g + 32, :],
                        ht8[32 * g : 32 * g + K2, :],
                        w2s[32 * g : 32 * g + K2, j : j + 512],
                        start=True,
                        stop=True,
                        tile_position=(32 * g, 32 * g),
                    )
                if m == GCOL // 512 - 1:
                    # split the final chunk's copy across both engines:
                    # hop1 waits on all of adj4, so its last copy is on
                    # the critical path to conv start
                    nc.vector.tensor_copy(adj4[:, j : j + 256], pa[:, 0:256])
                    nc.scalar.copy(adj4[:, j + 256 : j + 512], pa[:, 256:512])
                elif m % 2 == 0:
                    nc.vector.tensor_copy(adj4[:, j : j + 512], pa[:])
                else:
                    nc.scalar.copy(adj4[:, j : j + 512], pa[:])

            # PE warm-up filler: the hop1->fanout handoff leaves the PE
            # idle long enough for the HAM clock gate to drop to half
            # rate and re-ramp over the first ~13 conv chunks. These
            # junk matmuls (results never read) keep it busy — plain
            # untiled ones: tile_position'd matmuls don't register as
            # PE-busy to the HAM (k=8/8 never engaged during stage A).
            for w in range(15):
                pw = ptile([128, 512], name=f"warm{w}")
                nc.tensor.matmul(
                    pw[0:32, :],
                    ht8[0:K2, :],
                    w2s[0:K2, 0:512],
                    start=True,
                    stop=True,
                )

            # two-hop weight placement (see module docstring)
            wst = []
            for p in range(PAIRS):
                wstp = consts.tile([128, NT * 64], f16, name=f"wst{p}", tag=f"wst{p}")
                wst.append(wstp)
            # hop1: pair 0 on Q1 (empty by now -> immediate); pair 1 on
            # Q10 ahead of xp1's descriptors (it only gates conv p1 at
            # ~85us). Fan-outs: pair-0 halves split DVE/ACT so wblk[0]
            # is ready ~2 copies after hop1 lands.
            def hop1(b, eng):
                p, half = divmod(b, 2)
                q = half * 64
                return eng.dma_start(
                    out=wst[p][q : q + 64, :], in_=adj4[b : 128 : 32, :]
                )

            def fanout(b, eng):
                p, half = divmod(b, 2)
                q = half * 64
                dst = wblk[p][q : q + 64, :].rearrange(
                    "p (t co) -> p t co", co=128
                )[:, :, q : q + 64]
                src = wst[p][q : q + 64, :].rearrange("p (t co) -> p t co", co=64)
                if eng is nc.vector:
                    eng.tensor_copy(dst, src)
                else:
                    eng.copy(dst, src)

            hop1_last = None
            for b in range(BPC):
                hop1_last = hop1(b, nc.sync)
            load_x(0, range(2, XCH), after=hop1_last)
            load_x(1, range(XCH), after=hop1_last)
            fanout(0, nc.vector)
            fanout(1, nc.scalar)
            fanout(2, nc.vector)
            fanout(3, nc.scalar)

            # ---- stage B: per-pair conv, chunk-outer / tap-inner ----
            for p in range(PAIRS):
                xp3 = xps[p].rearrange("p (h w) -> p h w", w=WP)
                for g in range(8):
                    os = opool.tile([128, 2048], f16, name=f"os{p}_{g}", tag="os")
                    for j in range(4):
                        h0 = (g * 4 + j) * 4
                        po = ptile([128, 512], name=f"po{p}_{g}_{j}")
                        for t in range(NT):
                            kh, kw = divmod(t, 3)
                            nc.tensor.matmul(
                                po[:],
                                wblk[p][:, t * 128 : (t + 1) * 128],
                                xp3[:, h0 + kh : h0 + kh + 4, kw : kw + W],
                                start=(t == 0),
                                stop=(t == NT - 1),
                            )
                        if j % 2 == 0:
                            nc.vector.tensor_scalar_add(
                                os[:, j * 512 : (j + 1) * 512], po[:], cb_sb
                            )
                        else:
                            nc.scalar.add(
                                os[:, j * 512 : (j + 1) * 512], po[:], cb_sb
                            )
                        if p == PAIRS - 1 and g == 7 and j == 1:
                            # split the final group's store so most of it
                            # streams while the last chunks compute
                            nc.scalar.dma_start(
                                out=out_d.ap()[
                                    2 * p : 2 * p + 2, :, 16 * g : 16 * g + 8, :
                                ],
                                in_=os[:, 0:1024],
                            )
                    if p == PAIRS - 1 and g == 7:
                        # final store from the idle SP queue so it isn't
                        # serialized behind the last ACT bias-copy
                        nc.sync.dma_start(
                            out=out_d.ap()[
                                2 * p : 2 * p + 2, :, 16 * g + 8 : 16 * g + 16, :
                            ],
                            in_=os[:, 1024:2048],
                        )
                    else:
                        nc.scalar.dma_start(
                            out=out_d.ap()[
                                2 * p : 2 * p + 2, :, 16 * g : 16 * g + 16, :
                            ],
                            in_=os[:],
                        )

    nc.compile()
    return nc


def _get_nc():
    if "nc" not in _CACHE:
        _CACHE["nc"] = _build()
    return _CACHE["nc"]


def _prep(x, c, conv_w, conv_b, mlp_w1, mlp_b1, mlp_w2, mlp_b2):
    x = np.asarray(x, dtype=np.float32)
    c = np.asarray(c, dtype=np.float32)
    conv_w = np.asarray(conv_w, dtype=np.float32)
    conv_b = np.asarray(conv_b, dtype=np.float32)
    mlp_w1 = np.asarray(mlp_w1, dtype=np.float32)
    mlp_b1 = np.asarray(mlp_b1, dtype=np.float32)
    mlp_w2 = np.asarray(mlp_w2, dtype=np.float32)
    mlp_b2 = np.asarray(mlp_b2, dtype=np.float32)

    # padded fp16 x, flattened spatial
    xsp = np.zeros((B, CIN, HP, WP), dtype=np.float16)
    xsp[:, :, 1 : HP - 1, 1 : WP - 1] = x.astype(np.float16)
    xsp = xsp.reshape(B, CIN, HP * WP)

    # w2p[k, (ci, t, co)] = mlp_w2[k, co*576 + ci*9 + t]
    # row 16 = (mlp_b2 + conv_w), same permutation -> adj == full weight
    w2p = mlp_w2.reshape(MH, COUT, CIN, NT).transpose(0, 2, 3, 1).reshape(MH, WTOT)
    b2p = mlp_b2.reshape(COUT, CIN, NT).transpose(1, 2, 0)
    cwp = conv_w.reshape(COUT, CIN, NT).transpose(1, 2, 0)  # [ci, t, co]
    row16 = (b2p + cwp).reshape(1, WTOT)
    w2p = np.concatenate([w2p, row16], axis=0)  # [17, 36864]
    # 4-group pack: rows 32g..32g+17 carry cols [9216g, 9216(g+1))
    w2pk = np.zeros((128, GCOL), dtype=np.float16)
    for g in range(4):
        w2pk[32 * g : 32 * g + K2] = w2p[:, GCOL * g : GCOL * (g + 1)].astype(
            np.float16
        )

    # packed consts [128, 66] f32 (core-invariant part): cols 0-31 c'T
    # tiled 8x, 32-63 w1' zero-padded to 32, 64 b1 tiled per k-group,
    # 65 conv_b x2
    cstb = np.zeros((128, 66), dtype=np.float32)
    cstb[:CL, 32 : 32 + MH] = mlp_w1
    cstb[CL, 32 + MH] = 1.0
    for g in range(4):
        cstb[32 * g : 32 * g + MH, 64] = mlp_b1
    cstb[:, 65] = np.tile(conv_b, 2)

    in_maps = []
    for i in range(NCORES):
        sl = slice(i * BPC, (i + 1) * BPC)
        cst = cstb.copy()
        cst[:CL, 0:32] = np.tile(c[sl].T, (1, 8))
        cst[CL, 0:32] = 1.0
        in_maps.append(
            {
                "xsp": np.ascontiguousarray(xsp[sl]),
                "w2p": w2pk,
                "cst": cst,
            }
        )
    return in_maps


def _run(inputs, trace=False):
    from concourse.bass_utils import run_bass_kernel_spmd

    nc = _get_nc()
    in_maps = _prep(**inputs)
    res = run_bass_kernel_spmd(
        nc, in_maps, core_ids=list(range(NCORES)), trace=trace
    )
    out = np.concatenate(
        [res.results[i]["out"].astype(np.float32) for i in range(NCORES)], axis=0
    )
    return out, res


def kernel(**inputs):
    out, _ = _run(inputs, trace=False)
    return out
